# revision 50
# baseline (speedup 1.0000x reference)
"""Multi-head attention (B=16, T=1024, D=768, H=12) on 8 TRN2 NeuronCores.

Strategy: pure data parallelism over the batch — each core computes full MHA
for 2 batch elements. No collectives.

Device kernel design (per core, bf16 compute / fp32 accumulate):
  - Host pre-transposes x to xT[b] = x[b].T ([D, T]) and pre-packs all weights
    in SBUF-ready layouts, cast to bf16.
  - Heads are processed in pairs (2 x HS = 128 = partition width).
  - qT/kT ([128, T], head pair stacked on partitions) come from
    matmul(lhsT=W_pair[dchunk, 128], rhs=xT[dchunk, T]) accumulated over D.
  - S^T[s, t] per head via row-tiled (tile_position) K=64 matmuls packing both
    heads of a pair into the 128-row PE array concurrently.
  - exp via ScalarE activation (scale=1/sqrt(HS) folded in, no max subtraction:
    |S|/8 <= ~3 for this data, exp is safe in fp32->bf16).
  - O^T via COL-TILED matmuls: h0 at tile (0,0) (PE cols 0-63, out psum
    partitions 0-63), h1 at (0,64) (out partitions 64-127). Both co-execute
    and share one PSUM bank; h1's rows land directly at partitions 64-127 so
    the normalized output writes o_allT[0:128] with a single DVE multiply.
  - softmax denominators l[t] via col-tiled ones-stationary matmuls (M=64
    each, batched after the sc loop so the ones weights stay loaded):
    out[m, t] = sum_s es[s, t] for every m — the matmul natively broadcasts
    l across the partitions, so no cross-partition DMA/gpsimd traffic at all.
  - l -> 1/l via one reciprocal_approx_fast [128, TW] straight out of PSUM,
    normalization as a single DVE multiply per (pair, th).
  - y = O_all @ Wp + bp with lhsT = O_all^T; y stored bf16 (host casts back).

Scheduling (the engines execute their queues in order, so emission order IS
the schedule):
  - The attention inner loop is gated by the ScalarE exp cadence (~1us per
    128-column key chunk), leaving PE slack. All other PE work — the next
    pair's q/k projection, the v projection, the previous batch's output
    projection, and the previous (pair, th) unit's l/normalize tail — is
    emitted as fill thunks dripped into the sc steps of the loop (urgent
    fills have an intra-unit deadline; lazy fills absorb leftover slack).
  - The S->O software skew is 3 steps, and each unit's l-burst + reciprocal
    + normalize are deferred into the NEXT unit's early steps so the S/exp
    pipeline refills while the previous unit's tail completes.
  - q/k projections are emitted th-major (q-th0, k-th0, k-th1, q-th1) so a
    pair's first S matmul only depends on the first half of that work.
  - The next batch's x is prefetched at pair npair-3 and its v projection is
    scheduled into pairs npair-2/npair-1's slack (the lazy queue runs dry by
    then: only qk fills remain, exactly matching slack), which also clears
    the next batch's startup window.
  - DMA: the two queues (sync, gpsimd) are load-balanced at startup and
    ordered by first use so pair 0's pipeline starts ~7us in; warm-up bursts
    on the PE and ScalarE during the initial DMA wait keep the activity-
    based clock throttles open (without them everything runs ~20% slower).
"""

import os
from contextlib import ExitStack

import numpy as np
import ml_dtypes

import concourse.bacc as bacc
import concourse.bass as bass
import concourse.mybir as mybir
import concourse.tile as tile
from concourse.bass_utils import run_bass_kernel_spmd

BF16 = ml_dtypes.bfloat16

# Full problem dims
B, T_FULL, D_FULL, H, HS = 16, 1024, 768, 12, 64
N_CORES = 8
NB = B // N_CORES  # batch elements per core


def build_mha_nc(nb, t, d, npair, trn_type="TRN2", variant="full"):
    """Build the Bass program for `nb` batch elements, seq len `t`, model dim
    `d`, `npair` head pairs (each pair = 128 partition lanes)."""
    P = 128
    KC = d // P              # contraction chunks over model dim
    SC = t // P              # s (key position) chunks
    NTH = max(1, t // 512)   # output-column groups for S/O matmuls
    TW = t // NTH            # width of each group (<= 512)
    TC = t // P              # t row chunks for v/y
    D2 = d // 2              # y-proj free-dim split (<= 512 fp32 psum)
    dpair = 2 * HS           # 128
    scale = 1.0 / np.sqrt(HS)

    f32 = mybir.dt.float32
    bf16 = mybir.dt.bfloat16
    AF = mybir.ActivationFunctionType

    nc = bacc.Bacc(trn_type, target_bir_lowering=False, debug=False)

    xt_d = nc.dram_tensor("xt", [nb, d, t], bf16, kind="ExternalInput").ap()
    wq_d = nc.dram_tensor("wq", [P, npair, KC, dpair], bf16, kind="ExternalInput").ap()
    wk_d = nc.dram_tensor("wk", [P, npair, KC, dpair], bf16, kind="ExternalInput").ap()
    wv_d = nc.dram_tensor("wv", [P, KC, npair * dpair], bf16, kind="ExternalInput").ap()
    wp_d = nc.dram_tensor("wp", [P, KC, d], bf16, kind="ExternalInput").ap()
    bqk_d = nc.dram_tensor("bqk", [P, npair, 2], f32, kind="ExternalInput").ap()
    bv_d = nc.dram_tensor("bv", [P, npair, dpair], bf16, kind="ExternalInput").ap()
    bp_d = nc.dram_tensor("bp", [P, d], f32, kind="ExternalInput").ap()
    y_d = nc.dram_tensor("y", [nb, t, d], bf16, kind="ExternalOutput").ap()

    with TileOrExit(nc) as (tc, ctx):
        # ---- persistent weights (one bufs=1 pool; each tag allocated once) ----
        p_w = ctx.enter_context(tc.tile_pool(name="p_w", bufs=1))
        wq_sb = p_w.tile([P, npair, KC, dpair], bf16, tag="wq", name="wq_sb")
        wk_sb = p_w.tile([P, npair, KC, dpair], bf16, tag="wk", name="wk_sb")
        wv_sb = p_w.tile([P, KC, npair * dpair], bf16, tag="wv", name="wv_sb")
        wp_sb = p_w.tile([P, KC, d], bf16, tag="wp", name="wp_sb")
        bqk_sb = p_w.tile([P, npair, 2], f32, tag="bqk", name="bqk_sb")
        bv_sb = p_w.tile([P, npair, dpair], bf16, tag="bv", name="bv_sb")
        bp_sb = p_w.tile([P, d], f32, tag="bp", name="bp_sb")
        ones_sb = p_w.tile([P, HS], bf16, tag="ones", name="ones_sb")
        nc.vector.memset(ones_sb[:], 1.0)
        # weight loads: gpsimd queue carries everything needed in the first
        # ~15us (ordered by first use: pair-0 qk weights, v weights, the tiny
        # biases); later pairs' weights + wp/bp ride the sync queue behind
        # xt(b0). Both queues run in parallel so the first-pair pipeline can
        # start ~7us in.
        nc.gpsimd.dma_start(wq_sb[:, 0], wq_d[:, 0])
        nc.gpsimd.dma_start(bqk_sb[:], bqk_d)
        nc.gpsimd.dma_start(wk_sb[:, 0], wk_d[:, 0])
        def early_weight_dmas():
            # interleaved behind the odd xt chunks on the gpsimd queue
            for c in range(KC):
                nc.gpsimd.dma_start(wv_sb[:, c], wv_d[:, c])
            nc.gpsimd.dma_start(bv_sb[:], bv_d)
            for pr in (1, 2):
                nc.gpsimd.dma_start(wq_sb[:, pr], wq_d[:, pr])
                nc.gpsimd.dma_start(wk_sb[:, pr], wk_d[:, pr])

        def late_weight_dmas():
            for pr in (3, 4, 5):
                nc.sync.dma_start(wq_sb[:, pr], wq_d[:, pr])
                nc.sync.dma_start(wk_sb[:, pr], wk_d[:, pr])
            nc.sync.dma_start(wp_sb[:], wp_d)
            nc.sync.dma_start(bp_sb[:], bp_d)

        # ---- pools ----
        p_xt = ctx.enter_context(tc.tile_pool(name="p_xt", bufs=2))
        p_vall = ctx.enter_context(tc.tile_pool(name="p_vall", bufs=2))
        p_qk = ctx.enter_context(tc.tile_pool(name="p_qk", bufs=4))
        p_es = ctx.enter_context(tc.tile_pool(name="p_es", bufs=3))
        p_oall = ctx.enter_context(tc.tile_pool(name="p_oall", bufs=2))
        p_norm = ctx.enter_context(tc.tile_pool(name="p_norm", bufs=2))
        p_y = ctx.enter_context(tc.tile_pool(name="p_y", bufs=2))
        ps_s = ctx.enter_context(tc.tile_pool(name="ps_s", bufs=2, space="PSUM"))
        ps_O = ctx.enter_context(tc.tile_pool(name="ps_O", bufs=1, space="PSUM"))
        ps_L = ctx.enter_context(tc.tile_pool(name="ps_L", bufs=1, space="PSUM"))
        ps_m = ctx.enter_context(tc.tile_pool(name="ps_m", bufs=2, space="PSUM"))

        # Warm-up bursts during the initial DMA wait: keep the PE busy so the
        # HAM clock-gate opens to 2.4 GHz (and the chip-level activity
        # throttlers ramp) before real work arrives, and prime the Scalar
        # engine + its exp table.
        warm = p_norm.tile([P, TW], bf16, tag="warm", name="warm")
        warm2 = p_norm.tile([P, TW], bf16, tag="warm2", name="warm2")
        nc.vector.memset(warm[:], 0.0)
        wps = ps_m.tile([P, TW], f32, tag="m", name="wps")
        for i in range(6):
            nc.tensor.matmul(
                wps[:], lhsT=warm[:, 0:P], rhs=warm[:], start=(i == 0), stop=(i == 5)
            )
        # read wps so its ring slot releases immediately (an unread psum tile
        # would block the next "m"-tag allocation until pool close)
        nc.vector.tensor_copy(out=warm2[0:1, 0:1], in_=wps[0:1, 0:1])
        for i in range(4):
            nc.scalar.activation(out=warm2[:], in_=warm[:], func=AF.Exp, scale=1.0)

        nhalf = (npair + 2) // 3  # v-proj groups of <=3 pairs per psum tile

        def v_group(xt, v_all, tci, g):
            """One v-projection group: pairs 3g..3g+gn for one t-chunk."""
            gn = min(3, npair - 3 * g)
            psv = ps_m.tile([P, 3 * dpair], f32, tag="m", name="psv")
            for c in range(KC):
                nc.tensor.matmul(
                    psv[:, : gn * dpair],
                    lhsT=xt[:, c, tci * P : (tci + 1) * P],
                    rhs=wv_sb[:, c, 3 * g * dpair : (3 * g + gn) * dpair],
                    start=(c == 0),
                    stop=(c == KC - 1),
                )
            nc.vector.tensor_add(
                out=v_all[:, tci, 3 * g : 3 * g + gn, :],
                in0=psv[:, : gn * dpair].rearrange("p (r e) -> p r e", r=gn),
                in1=bv_sb[:, 3 * g : 3 * g + gn, :],
            )

        # ------------------------------------------------------------------
        # Software-pipelined emission: the attention inner loop is exp-gated
        # (~1us per sc step) leaving PE slack; all other PE work (q/k proj,
        # v proj, y proj, the previous unit's l/normalize tail) is emitted as
        # "fill" thunks dripped into the sc steps. `urgent` fills have a
        # deadline within the current unit (next pair's qk, this pair's v);
        # `lazy` fills (y proj of the previous batch, later v groups) absorb
        # the remaining slack.
        # ------------------------------------------------------------------
        urgent = []
        lazy = []

        def make_qk_thunks(xt, pr):
            """q/k projection for pair pr as 8 thunks, th-major and ordered
            by first use (q-th0, k-th0, k-th1, q-th1) so the first S matmul
            of the pair only waits on the first half of the work. Returns
            (qT, kT, thunks)."""
            qT = p_qk.tile([P, t], bf16, tag="qT", name="qT")
            kT = p_qk.tile([P, t], bf16, tag="kT", name="kT")
            thunks = []
            for w_sb, bj, dstT, th2 in (
                (wq_sb, 0, qT, 0),
                (wk_sb, 1, kT, 0),
                (wk_sb, 1, kT, 1),
                (wq_sb, 0, qT, 1),
            ):

                def mms(w_sb=w_sb, dstT=dstT, th2=th2, bj=bj):
                    psq = ps_m.tile([P, TW], f32, tag="m", name="psq")
                    for c in range(KC):
                        nc.tensor.matmul(
                            psq[:],
                            lhsT=w_sb[:, pr, c, :],
                            rhs=xt[:, c, th2 * TW : (th2 + 1) * TW],
                            start=(c == 0),
                            stop=(c == KC - 1),
                        )
                    nc.vector.tensor_scalar_add(
                        out=dstT[:, th2 * TW : (th2 + 1) * TW],
                        in0=psq[:],
                        scalar1=bqk_sb[:, pr, bj : bj + 1],
                    )

                thunks.append(mms)
            return qT, kT, thunks

        def make_y_thunks(o_allT, b, tci):
            """Output projection for one t-chunk as two thunks (one per
            half of the feature dim) so the psum ring pipelines MMs of one
            half with the bias-add drain of the other."""
            y_sb = p_y.tile([P, d], bf16, tag="y", name="y_sb")

            def run(j):
                psy = ps_m.tile([P, D2], f32, tag="m", name="psy")
                for c in range(KC):
                    nc.tensor.matmul(
                        psy[:],
                        lhsT=o_allT[:, c, tci * P : (tci + 1) * P],
                        rhs=wp_sb[:, c, j * D2 : (j + 1) * D2],
                        start=(c == 0),
                        stop=(c == KC - 1),
                    )
                nc.vector.tensor_add(
                    out=y_sb[:, j * D2 : (j + 1) * D2],
                    in0=psy[:],
                    in1=bp_sb[:, j * D2 : (j + 1) * D2],
                )
                if j == 1:
                    nc.sync.dma_start(
                        out=y_d[b, tci * P : (tci + 1) * P, :], in_=y_sb[:]
                    )

            return [lambda j=0: run(j), lambda j=1: run(j)]

        def make_l_thunk(es, psO, o_allT, pr, th):
            """Finish a unit: l matmuls (ones-stationary, broadcast across
            partitions), reciprocal, and the normalizing multiply."""

            def run():
                psL = ps_L.tile([P, TW], f32, tag="L", name="psL")
                for so in range(SC):
                    for h in range(2):
                        nc.tensor.matmul(
                            psL[64 * h : 64 * h + 64, :],
                            lhsT=ones_sb[:, 0:HS],
                            rhs=es[:, so, h, :],
                            start=(so == 0),
                            stop=(so == SC - 1),
                            tile_position=(0, 64 * h),
                        )
                linv = p_norm.tile([P, TW], f32, tag="linv", name="linv")
                if "norecip" in variant:
                    nc.vector.tensor_copy(out=linv[:], in_=psL[:])
                else:
                    nc.vector.reciprocal_approx_fast(out=linv[:], in_=psL[:])
                nc.vector.tensor_mul(
                    out=o_allT[:, pr, th * TW : (th + 1) * TW],
                    in0=psO[:],
                    in1=linv[:],
                )

            return run

        pending_l = None
        xt_tiles = {}
        v_ready = {}
        qk_ready = {}

        def emit_xt_dma(b, split=False):
            xt = p_xt.tile([P, KC, t], bf16, tag="xt", name="xt_sb")
            xt_src = xt_d[b].rearrange("(c p) t -> p c t", p=P)
            for c in range(KC):
                # at startup the last chunks ride the gpsimd queue so the two
                # DMA rings deliver x in parallel, in order, and pair 0's qk
                # starts sooner
                q = nc.gpsimd if (split and c >= KC // 2) else nc.sync
                q.dma_start(xt[:, c], xt_src[:, c])
            xt_tiles[b] = xt

        emit_xt_dma(0, split=True)
        early_weight_dmas()
        late_weight_dmas()
        for b in range(nb):
            xt = xt_tiles[b]
            # v_all[:, sc, pair, 0:64] = v_h0, [.., 64:128] = v_h1
            if b in v_ready:
                # this batch's v projection was already scheduled into the
                # previous batch's late-pair slack
                v_all = v_ready.pop(b)
            else:
                v_all = p_vall.tile([P, SC, npair, dpair], bf16, tag="vall", name="v_all")
                # group-0 v thunks feed pair 0's O matmuls (3 sc steps of
                # lead when drained 1/sc in pair 0 th 0)
                urgent.extend(
                    (lambda tci=tci: v_group(xt, v_all, tci, 0)) for tci in range(TC)
                )
                for g in range(1, nhalf):
                    lazy.extend(
                        (lambda tci=tci, g=g: v_group(xt, v_all, tci, g))
                        for tci in range(TC)
                    )
            o_allT = p_oall.tile([P, npair, t], bf16, tag="oall", name="o_allT")
            if b > 0:
                # previous batch's output projection fills this batch's slack
                for tci in range(TC):
                    lazy.extend(make_y_thunks(prev_o_allT, b - 1, tci))

            # pair 0's qk: pre-dripped into the previous batch's last pair
            # when possible; otherwise only the th0 halves precede the
            # attention and the th1 halves drip into the first unit
            if b in qk_ready:
                qT, kT = qk_ready.pop(b)
            else:
                qT, kT, tks = make_qk_thunks(xt, 0)
                tks[0]()
                tks[1]()
                urgent.insert(0, tks[3])
                urgent.insert(0, tks[2])

            for pr in range(npair):
                nxt = None
                if pr + 1 < npair:
                    nxt = make_qk_thunks(xt, pr + 1)
                elif b + 1 < nb:
                    # drip the next batch's pair-0 qk across this last pair's
                    # units, exactly like the mid-batch seam
                    nxt = make_qk_thunks(xt_tiles[b + 1], 0)
                    qk_ready[b + 1] = (nxt[0], nxt[1])
                for th in range(NTH):
                    # drip next pair's qk across BOTH th units: th0's mid-pair
                    # fill slots are otherwise dry (the l-tail latency shows
                    # up as ~280ns PE slivers there)
                    if nxt is not None:
                        half = len(nxt[2]) // 2
                        if th == 0:
                            urgent.extend(nxt[2][:half])
                        else:
                            urgent.extend(nxt[2][half:])
                    es = p_es.tile([P, SC, 2, TW], bf16, tag="es", name="es")
                    psO = ps_O.tile([P, TW], f32, tag="O", name="psO")
                    for sc in range(SC + 3):
                        if sc < SC:
                            ps = ps_s.tile([P, 2, TW], f32, tag="s", name="ps_s")
                            nc.tensor.matmul(
                                ps[:, 0, :],
                                lhsT=kT[0:64, sc * P : (sc + 1) * P],
                                rhs=qT[0:64, th * TW : (th + 1) * TW],
                                start=True,
                                stop=True,
                            )
                            nc.tensor.matmul(
                                ps[:, 1, :],
                                lhsT=kT[64:128, sc * P : (sc + 1) * P],
                                rhs=qT[64:128, th * TW : (th + 1) * TW],
                                start=True,
                                stop=True,
                                tile_position=(64, 0),
                            )
                            nc.scalar.activation(
                                out=es[:, sc, :, :], in_=ps[:], func=AF.Exp, scale=scale
                            )
                        # drip deferred work into the exp-gated slack
                        nfill = 1 if sc < SC else 2
                        for _ in range(nfill):
                            if urgent:
                                urgent.pop(0)()
                            elif lazy:
                                lazy.pop(0)()
                        # previous unit's l/normalize tail, after this unit's
                        # S pipeline has refilled the scalar engine
                        if sc == 2 and pending_l is not None:
                            pending_l()
                            pending_l = None
                        if sc >= 3:
                            so = sc - 3
                            for h in range(2):
                                nc.tensor.matmul(
                                    psO[64 * h : 64 * h + 64, :],
                                    lhsT=v_all[:, so, pr, 64 * h : 64 * h + 64],
                                    rhs=es[:, so, h, :],
                                    start=(so == 0),
                                    stop=(so == SC - 1),
                                    tile_position=(0, 64 * h),
                                )
                    pending_l = make_l_thunk(es, psO, o_allT, pr, th)
                if nxt is not None and pr + 1 < npair:
                    qT, kT = nxt[0], nxt[1]
                if pr == npair - 3 and b + 1 < nb:
                    emit_xt_dma(b + 1)
                if pr == npair - 2 and b + 1 < nb:
                    # the lazy queue runs dry by the late pairs (only qk fills
                    # remain, exactly matching slack): feed it the next
                    # batch's v projection, which also clears that batch's
                    # startup window
                    vn = p_vall.tile([P, SC, npair, dpair], bf16, tag="vall", name="v_all")
                    xtn = xt_tiles[b + 1]
                    for g in range(nhalf):
                        lazy.extend(
                            (lambda tci=tci, g=g, xtn=xtn, vn=vn: v_group(xtn, vn, tci, g))
                            for tci in range(TC)
                        )
                    v_ready[b + 1] = vn
            prev_o_allT = o_allT

        # ---- endgame: last unit's tail + last batch's output projection
        pending_l()
        pending_l = None
        for tci in range(TC):
            lazy.extend(make_y_thunks(prev_o_allT, nb - 1, tci))
        for tk in urgent + lazy:
            tk()
        urgent.clear()
        lazy.clear()

    nc.compile()
    return nc


class TileOrExit:
    """Combined TileContext + ExitStack context manager."""

    def __init__(self, nc):
        self.nc = nc
        self.ctx = ExitStack()
        self.tc = tile.TileContext(nc)

    def __enter__(self):
        self.ctx.__enter__()
        self.tc.__enter__()
        return self.tc, self.ctx

    def __exit__(self, *a):
        # close pools before TileContext exits scheduling
        self.ctx.__exit__(*a)
        return self.tc.__exit__(*a)


def prep_inputs(x, Wq, bq, Wk, bk, Wv, bv, Wp, bp, nb, npair):
    """Host-side packing into the DRAM layouts the device kernel expects.

    Returns a list of per-core input maps."""
    P = 128
    t = x.shape[1]
    d = x.shape[2]
    KC = d // P
    dpair = 2 * HS

    def to_bf(a):
        return np.ascontiguousarray(a).astype(BF16)

    # x^T per batch element
    xt = np.ascontiguousarray(x.transpose(0, 2, 1)).astype(BF16)  # [B, d, t]

    # wq/wk: [P, pair, c, 128] with cols 0:64 = head 2p, 64:128 = head 2p+1
    def pack_qk(W):
        # W: [H, d, HS] -> [pair, 2, KC, P, HS] -> [P, pair, KC, 2*HS]
        w = W.reshape(npair, 2, KC, P, HS)
        w = w.transpose(3, 0, 2, 1, 4).reshape(P, npair, KC, dpair)
        return to_bf(w)

    wq = pack_qk(Wq)
    wk = pack_qk(Wk)
    wv = pack_qk(Wv).transpose(0, 2, 1, 3).reshape(P, KC, npair * dpair)
    wv = np.ascontiguousarray(wv)  # [P, c, pair*128]
    # wp: [P, c, d]
    wp = to_bf(Wp.reshape(KC, P, d).transpose(1, 0, 2))
    # bqk: [P, pair, 2] fp32: partition = pair-stacked head dims
    bqk = np.stack(
        [bq.reshape(npair, dpair), bk.reshape(npair, dpair)], axis=-1
    )  # [pair, 128, 2]
    bqk = np.ascontiguousarray(bqk.transpose(1, 0, 2)).astype(np.float32)  # [P, pair, 2]
    # bv broadcast along t partitions: [P, pair, 128]
    bv_bc = np.broadcast_to(bv.reshape(1, npair, dpair), (P, npair, dpair))
    bv_bc = to_bf(bv_bc)
    # bp broadcast: [P, d] fp32
    bp_bc = np.ascontiguousarray(np.broadcast_to(bp.reshape(1, d), (P, d))).astype(
        np.float32
    )

    weights = {
        "wq": wq,
        "wk": wk,
        "wv": wv,
        "wp": wp,
        "bqk": bqk,
        "bv": bv_bc,
        "bp": bp_bc,
    }
    n_cores = x.shape[0] // nb
    in_maps = []
    for i in range(n_cores):
        m = dict(weights)
        m["xt"] = np.ascontiguousarray(xt[i * nb : (i + 1) * nb])
        in_maps.append(m)
    return in_maps


_NC_CACHE = {}
LAST_RESULT = {}


def kernel(x, Wq, bq, Wk, bk, Wv, bv, Wp, bp, _trace=False):
    x = np.asarray(x, dtype=np.float32)
    Wq, bq = np.asarray(Wq, np.float32), np.asarray(bq, np.float32)
    Wk, bk = np.asarray(Wk, np.float32), np.asarray(bk, np.float32)
    Wv, bv = np.asarray(Wv, np.float32), np.asarray(bv, np.float32)
    Wp, bp = np.asarray(Wp, np.float32), np.asarray(bp, np.float32)

    npair = H // 2
    key = ("full", NB, T_FULL, D_FULL, npair)
    if key not in _NC_CACHE:
        _NC_CACHE[key] = build_mha_nc(NB, T_FULL, D_FULL, npair)
    nc = _NC_CACHE[key]

    in_maps = prep_inputs(x, Wq, bq, Wk, bk, Wv, bv, Wp, bp, NB, npair)
    res = run_bass_kernel_spmd(
        nc, in_maps, core_ids=list(range(N_CORES)), trace=_trace
    )
    LAST_RESULT["exec_time_ns"] = res.exec_time_ns
    LAST_RESULT["res"] = res
    outs = [res.results[i]["y"] for i in range(N_CORES)]
    return np.concatenate(outs, axis=0).astype(np.float32)


# revision 51
# speedup vs baseline: 1.1971x; 1.1971x over previous
"""Multi-head attention (B=16, T=1024, D=768, H=12) on 8 TRN2 NeuronCores.

Strategy: pure data parallelism over the batch — each core computes full MHA
for 2 batch elements. No collectives.

Device kernel design (per core, bf16 compute / fp32 accumulate):
  - Host pre-transposes x to xT[b] = x[b].T ([D, T]) and pre-packs all weights
    in SBUF-ready layouts, cast to bf16.
  - Heads are processed in pairs (2 x HS = 128 = partition width).
  - qT/kT ([128, T], head pair stacked on partitions) come from
    matmul(lhsT=W_pair[dchunk, 128], rhs=xT[dchunk, T]) accumulated over D.
  - S^T[s, t] per head via row-tiled (tile_position) K=64 matmuls packing both
    heads of a pair into the 128-row PE array concurrently.
  - exp via ScalarE activation (scale=1/sqrt(HS) folded in, no max subtraction:
    |S|/8 <= ~3 for this data, exp is safe in fp32->bf16).
  - O^T via COL-TILED matmuls: h0 at tile (0,0) (PE cols 0-63, out psum
    partitions 0-63), h1 at (0,64) (out partitions 64-127). Both co-execute
    and share one PSUM bank; h1's rows land directly at partitions 64-127 so
    the normalized output writes o_allT[0:128] with a single DVE multiply.
  - softmax denominators l[t] via col-tiled ones-stationary matmuls (M=64
    each, batched after the sc loop so the ones weights stay loaded):
    out[m, t] = sum_s es[s, t] for every m — the matmul natively broadcasts
    l across the partitions, so no cross-partition DMA/gpsimd traffic at all.
  - l -> 1/l via one reciprocal_approx_fast [128, TW] straight out of PSUM,
    normalization as a single DVE multiply per (pair, th).
  - y = O_all @ Wp + bp with lhsT = O_all^T; y stored bf16 (host casts back).

Scheduling (the engines execute their queues in order, so emission order IS
the schedule):
  - The attention inner loop is gated by the ScalarE exp cadence (~1us per
    128-column key chunk), leaving PE slack. All other PE work — the next
    pair's q/k projection, the v projection, the previous batch's output
    projection, and the previous (pair, th) unit's l/normalize tail — is
    emitted as fill thunks dripped into the sc steps of the loop (urgent
    fills have an intra-unit deadline; lazy fills absorb leftover slack).
  - The S->O software skew is 3 steps, and each unit's l-burst + reciprocal
    + normalize are deferred into the NEXT unit's early steps so the S/exp
    pipeline refills while the previous unit's tail completes.
  - q/k projections are emitted th-major (q-th0, k-th0, k-th1, q-th1) so a
    pair's first S matmul only depends on the first half of that work.
  - The next batch's x is prefetched at pair npair-3 and its v projection is
    scheduled into pairs npair-2/npair-1's slack (the lazy queue runs dry by
    then: only qk fills remain, exactly matching slack), which also clears
    the next batch's startup window.
  - DMA: the two queues (sync, gpsimd) are load-balanced at startup and
    ordered by first use so pair 0's pipeline starts ~7us in; warm-up bursts
    on the PE and ScalarE during the initial DMA wait keep the activity-
    based clock throttles open (without them everything runs ~20% slower).
"""

import os
from contextlib import ExitStack

import numpy as np
import ml_dtypes

import concourse.bacc as bacc
import concourse.bass as bass
import concourse.mybir as mybir
import concourse.tile as tile
from concourse.bass_utils import run_bass_kernel_spmd

BF16 = ml_dtypes.bfloat16

# Full problem dims
B, T_FULL, D_FULL, H, HS = 16, 1024, 768, 12, 64
N_CORES = 8
NB = B // N_CORES  # batch elements per core


def build_mha_nc(nb, t, d, npair, trn_type="TRN2", variant="full"):
    """Build the Bass program for `nb` batch elements, seq len `t`, model dim
    `d`, `npair` head pairs (each pair = 128 partition lanes)."""
    P = 128
    KC = d // P              # contraction chunks over model dim
    SC = t // P              # s (key position) chunks
    NTH = max(1, t // 512)   # output-column groups for S/O matmuls
    TW = t // NTH            # width of each group (<= 512)
    TC = t // P              # t row chunks for v/y
    D2 = d // 2              # y-proj free-dim split (<= 512 fp32 psum)
    dpair = 2 * HS           # 128
    scale = 1.0 / np.sqrt(HS)

    f32 = mybir.dt.float32
    bf16 = mybir.dt.bfloat16
    AF = mybir.ActivationFunctionType

    nc = bacc.Bacc(trn_type, target_bir_lowering=False, debug=False)

    xt_d = nc.dram_tensor("xt", [nb, d, t], bf16, kind="ExternalInput").ap()
    wq_d = nc.dram_tensor("wq", [P, npair, KC, dpair], bf16, kind="ExternalInput").ap()
    wk_d = nc.dram_tensor("wk", [P, npair, KC, dpair], bf16, kind="ExternalInput").ap()
    wv_d = nc.dram_tensor("wv", [P, KC, npair * dpair], bf16, kind="ExternalInput").ap()
    wp_d = nc.dram_tensor("wp", [P, KC, d], bf16, kind="ExternalInput").ap()
    bqk_d = nc.dram_tensor("bqk", [P, npair, 2], f32, kind="ExternalInput").ap()
    bv_d = nc.dram_tensor("bv", [P, npair, dpair], bf16, kind="ExternalInput").ap()
    bp_d = nc.dram_tensor("bp", [P, d], f32, kind="ExternalInput").ap()
    y_d = nc.dram_tensor("y", [nb, t, d], bf16, kind="ExternalOutput").ap()

    with TileOrExit(nc) as (tc, ctx):
        # ---- persistent weights (one bufs=1 pool; each tag allocated once) ----
        p_w = ctx.enter_context(tc.tile_pool(name="p_w", bufs=1))
        wq_sb = p_w.tile([P, npair, KC, dpair], bf16, tag="wq", name="wq_sb")
        wk_sb = p_w.tile([P, npair, KC, dpair], bf16, tag="wk", name="wk_sb")
        wv_sb = p_w.tile([P, KC, npair * dpair], bf16, tag="wv", name="wv_sb")
        wp_sb = p_w.tile([P, KC, d], bf16, tag="wp", name="wp_sb")
        bqk_sb = p_w.tile([P, npair, 2], f32, tag="bqk", name="bqk_sb")
        bv_sb = p_w.tile([P, npair, dpair], bf16, tag="bv", name="bv_sb")
        bp_sb = p_w.tile([P, d], f32, tag="bp", name="bp_sb")
        ones_sb = p_w.tile([P, HS], bf16, tag="ones", name="ones_sb")
        nc.vector.memset(ones_sb[:], 1.0)
        # weight loads: gpsimd queue carries everything needed in the first
        # ~15us (ordered by first use: pair-0 qk weights, v weights, the tiny
        # biases); later pairs' weights + wp/bp ride the sync queue behind
        # xt(b0). Both queues run in parallel so the first-pair pipeline can
        # start ~7us in.
        nc.gpsimd.dma_start(wq_sb[:, 0], wq_d[:, 0])
        nc.gpsimd.dma_start(bqk_sb[:], bqk_d)
        nc.gpsimd.dma_start(wk_sb[:, 0], wk_d[:, 0])
        def early_weight_dmas():
            # interleaved behind the odd xt chunks on the gpsimd queue
            for c in range(KC):
                nc.gpsimd.dma_start(wv_sb[:, c], wv_d[:, c])
            nc.gpsimd.dma_start(bv_sb[:], bv_d)
            for pr in (1, 2):
                nc.gpsimd.dma_start(wq_sb[:, pr], wq_d[:, pr])
                nc.gpsimd.dma_start(wk_sb[:, pr], wk_d[:, pr])

        def late_weight_dmas():
            for pr in (3, 4, 5):
                nc.sync.dma_start(wq_sb[:, pr], wq_d[:, pr])
                nc.sync.dma_start(wk_sb[:, pr], wk_d[:, pr])
            nc.sync.dma_start(wp_sb[:], wp_d)
            nc.sync.dma_start(bp_sb[:], bp_d)

        # ---- pools ----
        p_xt = ctx.enter_context(tc.tile_pool(name="p_xt", bufs=2))
        p_vall = ctx.enter_context(tc.tile_pool(name="p_vall", bufs=2))
        p_qk = ctx.enter_context(tc.tile_pool(name="p_qk", bufs=4))
        p_es = ctx.enter_context(tc.tile_pool(name="p_es", bufs=3))
        p_oall = ctx.enter_context(tc.tile_pool(name="p_oall", bufs=2))
        p_norm = ctx.enter_context(tc.tile_pool(name="p_norm", bufs=2))
        p_y = ctx.enter_context(tc.tile_pool(name="p_y", bufs=2))
        ps_s = ctx.enter_context(tc.tile_pool(name="ps_s", bufs=2, space="PSUM"))
        ps_O = ctx.enter_context(tc.tile_pool(name="ps_O", bufs=1, space="PSUM"))
        ps_L = ctx.enter_context(tc.tile_pool(name="ps_L", bufs=1, space="PSUM"))
        ps_m = ctx.enter_context(tc.tile_pool(name="ps_m", bufs=2, space="PSUM"))

        # Warm-up bursts during the initial DMA wait: keep the PE busy so the
        # HAM clock-gate opens to 2.4 GHz (and the chip-level activity
        # throttlers ramp) before real work arrives, and prime the Scalar
        # engine + its exp table.
        warm = p_norm.tile([P, TW], bf16, tag="warm", name="warm")
        warm2 = p_norm.tile([P, TW], bf16, tag="warm2", name="warm2")
        nc.vector.memset(warm[:], 0.0)
        wps = ps_m.tile([P, TW], f32, tag="m", name="wps")
        for i in range(6):
            nc.tensor.matmul(
                wps[:], lhsT=warm[:, 0:P], rhs=warm[:], start=(i == 0), stop=(i == 5)
            )
        # read wps so its ring slot releases immediately (an unread psum tile
        # would block the next "m"-tag allocation until pool close)
        nc.vector.tensor_copy(out=warm2[0:1, 0:1], in_=wps[0:1, 0:1])
        for i in range(4):
            nc.scalar.activation(out=warm2[:], in_=warm[:], func=AF.Exp, scale=1.0)

        nhalf = (npair + 2) // 3  # v-proj groups of <=3 pairs per psum tile

        def v_group(xt, v_all, tci, g):
            """One v-projection group: pairs 3g..3g+gn for one t-chunk."""
            gn = min(3, npair - 3 * g)
            psv = ps_m.tile([P, 3 * dpair], f32, tag="m", name="psv")
            for c in range(KC):
                nc.tensor.matmul(
                    psv[:, : gn * dpair],
                    lhsT=xt[:, c, tci * P : (tci + 1) * P],
                    rhs=wv_sb[:, c, 3 * g * dpair : (3 * g + gn) * dpair],
                    start=(c == 0),
                    stop=(c == KC - 1),
                )
            nc.vector.tensor_add(
                out=v_all[:, tci, 3 * g : 3 * g + gn, :],
                in0=psv[:, : gn * dpair].rearrange("p (r e) -> p r e", r=gn),
                in1=bv_sb[:, 3 * g : 3 * g + gn, :],
            )

        # ------------------------------------------------------------------
        # Software-pipelined emission: the attention inner loop is exp-gated
        # (~1us per sc step) leaving PE slack; all other PE work (q/k proj,
        # v proj, y proj, the previous unit's l/normalize tail) is emitted as
        # "fill" thunks dripped into the sc steps. `urgent` fills have a
        # deadline within the current unit (next pair's qk, this pair's v);
        # `lazy` fills (y proj of the previous batch, later v groups) absorb
        # the remaining slack.
        # ------------------------------------------------------------------
        urgent = []
        lazy = []

        def make_qk_thunks(xt, pr):
            """q/k projection for pair pr as 8 thunks, th-major and ordered
            by first use (q-th0, k-th0, k-th1, q-th1) so the first S matmul
            of the pair only waits on the first half of the work. Returns
            (qT, kT, thunks)."""
            qT = p_qk.tile([P, t], bf16, tag="qT", name="qT")
            kT = p_qk.tile([P, t], bf16, tag="kT", name="kT")
            thunks = []
            for w_sb, bj, dstT, th2 in (
                (wq_sb, 0, qT, 0),
                (wk_sb, 1, kT, 0),
                (wk_sb, 1, kT, 1),
                (wq_sb, 0, qT, 1),
            ):

                def mms(w_sb=w_sb, dstT=dstT, th2=th2, bj=bj):
                    psq = ps_m.tile([P, TW], f32, tag="m", name="psq")
                    for c in range(KC):
                        nc.tensor.matmul(
                            psq[:],
                            lhsT=w_sb[:, pr, c, :],
                            rhs=xt[:, c, th2 * TW : (th2 + 1) * TW],
                            start=(c == 0),
                            stop=(c == KC - 1),
                        )
                    nc.vector.tensor_scalar_add(
                        out=dstT[:, th2 * TW : (th2 + 1) * TW],
                        in0=psq[:],
                        scalar1=bqk_sb[:, pr, bj : bj + 1],
                    )

                thunks.append(mms)
            return qT, kT, thunks

        def make_y_thunks(o_allT, b, tci):
            """Output projection for one t-chunk as two thunks (one per
            half of the feature dim) so the psum ring pipelines MMs of one
            half with the bias-add drain of the other."""
            y_sb = p_y.tile([P, d], bf16, tag="y", name="y_sb")

            def run(j):
                psy = ps_m.tile([P, D2], f32, tag="m", name="psy")
                for c in range(KC):
                    nc.tensor.matmul(
                        psy[:],
                        lhsT=o_allT[:, c, tci * P : (tci + 1) * P],
                        rhs=wp_sb[:, c, j * D2 : (j + 1) * D2],
                        start=(c == 0),
                        stop=(c == KC - 1),
                    )
                nc.vector.tensor_add(
                    out=y_sb[:, j * D2 : (j + 1) * D2],
                    in0=psy[:],
                    in1=bp_sb[:, j * D2 : (j + 1) * D2],
                )
                if j == 1:
                    nc.sync.dma_start(
                        out=y_d[b, tci * P : (tci + 1) * P, :], in_=y_sb[:]
                    )

            return [lambda j=0: run(j), lambda j=1: run(j)]

        def make_l_thunk(es, psO, o_allT, pr, th):
            """Finish a unit: l matmuls (ones-stationary, broadcast across
            partitions), reciprocal, and the normalizing multiply."""

            def run():
                psL = ps_L.tile([P, TW], f32, tag="L", name="psL")
                for so in range(SC):
                    for h in range(2):
                        nc.tensor.matmul(
                            psL[64 * h : 64 * h + 64, :],
                            lhsT=ones_sb[:, 0:HS],
                            rhs=es[:, so, h, :],
                            start=(so == 0),
                            stop=(so == SC - 1),
                            tile_position=(0, 64 * h),
                        )
                linv = p_norm.tile([P, TW], f32, tag="linv", name="linv")
                if "norecip" in variant:
                    nc.vector.tensor_copy(out=linv[:], in_=psL[:])
                else:
                    nc.vector.reciprocal_approx_fast(out=linv[:], in_=psL[:])
                nc.vector.tensor_mul(
                    out=o_allT[:, pr, th * TW : (th + 1) * TW],
                    in0=psO[:],
                    in1=linv[:],
                )

            return run

        pending_l = None
        xt_tiles = {}
        v_ready = {}

        def emit_xt_dma(b, split=False):
            xt = p_xt.tile([P, KC, t], bf16, tag="xt", name="xt_sb")
            xt_src = xt_d[b].rearrange("(c p) t -> p c t", p=P)
            for c in range(KC):
                # at startup the last chunks ride the gpsimd queue so the two
                # DMA rings deliver x in parallel, in order, and pair 0's qk
                # starts sooner
                q = nc.gpsimd if (split and c >= KC // 2) else nc.sync
                q.dma_start(xt[:, c], xt_src[:, c])
            xt_tiles[b] = xt

        emit_xt_dma(0, split=True)
        early_weight_dmas()
        late_weight_dmas()
        for b in range(nb):
            xt = xt_tiles[b]
            # v_all[:, sc, pair, 0:64] = v_h0, [.., 64:128] = v_h1
            if b in v_ready:
                # this batch's v projection was already scheduled into the
                # previous batch's late-pair slack
                v_all = v_ready.pop(b)
            else:
                v_all = p_vall.tile([P, SC, npair, dpair], bf16, tag="vall", name="v_all")
                # group-0 v thunks feed pair 0's O matmuls (3 sc steps of
                # lead when drained 1/sc in pair 0 th 0)
                urgent.extend(
                    (lambda tci=tci: v_group(xt, v_all, tci, 0)) for tci in range(TC)
                )
                for g in range(1, nhalf):
                    lazy.extend(
                        (lambda tci=tci, g=g: v_group(xt, v_all, tci, g))
                        for tci in range(TC)
                    )
            o_allT = p_oall.tile([P, npair, t], bf16, tag="oall", name="o_allT")
            if b > 0:
                # previous batch's output projection fills this batch's slack
                for tci in range(TC):
                    lazy.extend(make_y_thunks(prev_o_allT, b - 1, tci))

            # pair 0's qk: only the th0 halves must precede the attention;
            # the th1 halves drip into the first unit's early sc slots
            qT, kT, tks = make_qk_thunks(xt, 0)
            tks[0]()
            tks[1]()
            urgent.insert(0, tks[3])
            urgent.insert(0, tks[2])

            for pr in range(npair):
                nxt = None
                if pr + 1 < npair:
                    nxt = make_qk_thunks(xt, pr + 1)
                for th in range(NTH):
                    # drip next pair's qk across BOTH th units: th0's mid-pair
                    # fill slots are otherwise dry (the l-tail latency shows
                    # up as ~280ns PE slivers there)
                    if nxt is not None:
                        half = len(nxt[2]) // 2
                        if th == 0:
                            urgent.extend(nxt[2][:half])
                        else:
                            urgent.extend(nxt[2][half:])
                    es = p_es.tile([P, SC, 2, TW], bf16, tag="es", name="es")
                    psO = ps_O.tile([P, TW], f32, tag="O", name="psO")
                    for sc in range(SC + 3):
                        if sc < SC:
                            ps = ps_s.tile([P, 2, TW], f32, tag="s", name="ps_s")
                            nc.tensor.matmul(
                                ps[:, 0, :],
                                lhsT=kT[0:64, sc * P : (sc + 1) * P],
                                rhs=qT[0:64, th * TW : (th + 1) * TW],
                                start=True,
                                stop=True,
                            )
                            nc.tensor.matmul(
                                ps[:, 1, :],
                                lhsT=kT[64:128, sc * P : (sc + 1) * P],
                                rhs=qT[64:128, th * TW : (th + 1) * TW],
                                start=True,
                                stop=True,
                                tile_position=(64, 0),
                            )
                            nc.scalar.activation(
                                out=es[:, sc, :, :], in_=ps[:], func=AF.Exp, scale=scale
                            )
                        # drip deferred work into the exp-gated slack
                        nfill = 1 if sc < SC else 2
                        for _ in range(nfill):
                            if urgent:
                                urgent.pop(0)()
                            elif lazy:
                                lazy.pop(0)()
                        # previous unit's l/normalize tail, after this unit's
                        # S pipeline has refilled the scalar engine
                        if sc == 2 and pending_l is not None:
                            pending_l()
                            pending_l = None
                        if sc >= 3:
                            so = sc - 3
                            for h in range(2):
                                nc.tensor.matmul(
                                    psO[64 * h : 64 * h + 64, :],
                                    lhsT=v_all[:, so, pr, 64 * h : 64 * h + 64],
                                    rhs=es[:, so, h, :],
                                    start=(so == 0),
                                    stop=(so == SC - 1),
                                    tile_position=(0, 64 * h),
                                )
                    pending_l = make_l_thunk(es, psO, o_allT, pr, th)
                if nxt is not None:
                    qT, kT = nxt[0], nxt[1]
                if pr == npair - 3 and b + 1 < nb:
                    emit_xt_dma(b + 1)
                if pr == npair - 2 and b + 1 < nb:
                    # the lazy queue runs dry by the late pairs (only qk fills
                    # remain, exactly matching slack): feed it the next
                    # batch's v projection, which also clears that batch's
                    # startup window
                    vn = p_vall.tile([P, SC, npair, dpair], bf16, tag="vall", name="v_all")
                    xtn = xt_tiles[b + 1]
                    for g in range(nhalf):
                        lazy.extend(
                            (lambda tci=tci, g=g, xtn=xtn, vn=vn: v_group(xtn, vn, tci, g))
                            for tci in range(TC)
                        )
                    v_ready[b + 1] = vn
            prev_o_allT = o_allT

        # ---- endgame: last unit's tail + last batch's output projection
        pending_l()
        pending_l = None
        for tci in range(TC):
            lazy.extend(make_y_thunks(prev_o_allT, nb - 1, tci))
        for tk in urgent + lazy:
            tk()
        urgent.clear()
        lazy.clear()

    nc.compile()
    return nc


class TileOrExit:
    """Combined TileContext + ExitStack context manager."""

    def __init__(self, nc):
        self.nc = nc
        self.ctx = ExitStack()
        self.tc = tile.TileContext(nc)

    def __enter__(self):
        self.ctx.__enter__()
        self.tc.__enter__()
        return self.tc, self.ctx

    def __exit__(self, *a):
        # close pools before TileContext exits scheduling
        self.ctx.__exit__(*a)
        return self.tc.__exit__(*a)


def prep_inputs(x, Wq, bq, Wk, bk, Wv, bv, Wp, bp, nb, npair):
    """Host-side packing into the DRAM layouts the device kernel expects.

    Returns a list of per-core input maps."""
    P = 128
    t = x.shape[1]
    d = x.shape[2]
    KC = d // P
    dpair = 2 * HS

    def to_bf(a):
        return np.ascontiguousarray(a).astype(BF16)

    # x^T per batch element
    xt = np.ascontiguousarray(x.transpose(0, 2, 1)).astype(BF16)  # [B, d, t]

    # wq/wk: [P, pair, c, 128] with cols 0:64 = head 2p, 64:128 = head 2p+1
    def pack_qk(W):
        # W: [H, d, HS] -> [pair, 2, KC, P, HS] -> [P, pair, KC, 2*HS]
        w = W.reshape(npair, 2, KC, P, HS)
        w = w.transpose(3, 0, 2, 1, 4).reshape(P, npair, KC, dpair)
        return to_bf(w)

    wq = pack_qk(Wq)
    wk = pack_qk(Wk)
    wv = pack_qk(Wv).transpose(0, 2, 1, 3).reshape(P, KC, npair * dpair)
    wv = np.ascontiguousarray(wv)  # [P, c, pair*128]
    # wp: [P, c, d]
    wp = to_bf(Wp.reshape(KC, P, d).transpose(1, 0, 2))
    # bqk: [P, pair, 2] fp32: partition = pair-stacked head dims
    bqk = np.stack(
        [bq.reshape(npair, dpair), bk.reshape(npair, dpair)], axis=-1
    )  # [pair, 128, 2]
    bqk = np.ascontiguousarray(bqk.transpose(1, 0, 2)).astype(np.float32)  # [P, pair, 2]
    # bv broadcast along t partitions: [P, pair, 128]
    bv_bc = np.broadcast_to(bv.reshape(1, npair, dpair), (P, npair, dpair))
    bv_bc = to_bf(bv_bc)
    # bp broadcast: [P, d] fp32
    bp_bc = np.ascontiguousarray(np.broadcast_to(bp.reshape(1, d), (P, d))).astype(
        np.float32
    )

    weights = {
        "wq": wq,
        "wk": wk,
        "wv": wv,
        "wp": wp,
        "bqk": bqk,
        "bv": bv_bc,
        "bp": bp_bc,
    }
    n_cores = x.shape[0] // nb
    in_maps = []
    for i in range(n_cores):
        m = dict(weights)
        m["xt"] = np.ascontiguousarray(xt[i * nb : (i + 1) * nb])
        in_maps.append(m)
    return in_maps


_NC_CACHE = {}
LAST_RESULT = {}


def kernel(x, Wq, bq, Wk, bk, Wv, bv, Wp, bp, _trace=False):
    x = np.asarray(x, dtype=np.float32)
    Wq, bq = np.asarray(Wq, np.float32), np.asarray(bq, np.float32)
    Wk, bk = np.asarray(Wk, np.float32), np.asarray(bk, np.float32)
    Wv, bv = np.asarray(Wv, np.float32), np.asarray(bv, np.float32)
    Wp, bp = np.asarray(Wp, np.float32), np.asarray(bp, np.float32)

    npair = H // 2
    key = ("full", NB, T_FULL, D_FULL, npair)
    if key not in _NC_CACHE:
        _NC_CACHE[key] = build_mha_nc(NB, T_FULL, D_FULL, npair)
    nc = _NC_CACHE[key]

    in_maps = prep_inputs(x, Wq, bq, Wk, bk, Wv, bv, Wp, bp, NB, npair)
    res = run_bass_kernel_spmd(
        nc, in_maps, core_ids=list(range(N_CORES)), trace=_trace
    )
    LAST_RESULT["exec_time_ns"] = res.exec_time_ns
    LAST_RESULT["res"] = res
    outs = [res.results[i]["y"] for i in range(N_CORES)]
    return np.concatenate(outs, axis=0).astype(np.float32)
